# revision 32
# baseline (speedup 1.0000x reference)
"""Dilated attention (banded local-window attention) for Trainium2.

Problem: q,k,v [1, 16, 4096, 64] fp32; dilation r=2, window 128 (band |i-j|<=64
within each of the 2 strided subsequences of length 2048 per head).

Sharding: 16 heads -> 8 cores x 2 heads. Host work is a single fp32->bf16 cast
per tensor (ml_dtypes, ~10ms each); ALL relayout happens on device:

- Q^T/K^T: xbar DMA-transposes turn DRAM-natural [2048, (r d)=128] bf16 into
  SBUF [(r d), 2048] d-major layout; offset r=0 lands on partitions 0:64 and
  r=1 on 64:128 (the dilation de-interleave falls out of the transpose since
  (r d) becomes the partition dim). K^T lands at column offset 64 inside a
  [128, 2176] tile whose edge columns are zeroed. The transposes are SP-issued
  (ACT-issued DmaTranspose corrupts on this runtime) and serialize against
  all other DMA traffic, so they run as one tight chain before the V loads.
- V: the shifted window layout vsh[p, t, r, :64] = v[2*(128t - 64 + p) + r]
  (+ ones column for the row-sum trick) is built with strided DMAs plus edge
  memsets.

Per block, queries are tiled in 16 tiles of 128; each tile attends to a
256-key window. Scores are computed transposed (S^T[jj, i]) so probabilities
come out pre-transposed for the PV matmul. Softmax skips the max-subtraction
(scores ~ N(0,1) after the 1/8 scale) and folds the scale into the ScalarE
exp. The band mask is a 0/1 bf16 multiply after exp. Row sums come from the
ones-column: out = (P@[V|1])[:,:64] / (P@[V|1])[:,64].

Emission is phase-split so the ScalarE exp chain (the serial bottleneck)
never stalls: QK+exp+mask for both heads runs while V still loads; all PV
work is pinned after the V-load gates.
"""

import sys

for _p in ("/opt/trn_rl_repo", "/opt/trn_rl_repo/concourse"):
    if _p not in sys.path:
        sys.path.insert(0, _p)

import numpy as np
import ml_dtypes

import concourse.bass as bass
import concourse.mybir as mybir
import concourse.tile as tile
from concourse import bacc
from concourse.bass_utils import run_bass_kernel_spmd
from concourse.tile_rust import add_dep_helper

N_CORES = 8
B, H, S, D = 1, 16, 4096, 64
R = 2                      # dilation rate
NSEQ = S // R              # 2048 per-offset sequence length
HALF = 64                  # window//2
NT = NSEQ // 128           # 16 query tiles per block
HPC = H // N_CORES         # heads per core = 2

F32 = mybir.dt.float32
BF16 = mybir.dt.bfloat16


def _dummy(nc, dmy, col, a, b, after=None):
    """One PE-proc absorber: a tiny dummy matmul reading cells a/b so the PE
    sequencer observes their DMA-queue semaphores here (<=2 waits). The
    scheduler floats unpinned dummies past the real matmuls (leaving the DMA
    waits guarding nothing), so callers must pin consumers after these via
    add_dep_helper, and `after` keeps the dummy from stalling the PE early."""
    dm = nc.tensor.matmul(dmy[0:1, col : col + 1], lhsT=a, rhs=b,
                          start=True, stop=True)
    if after is not None:
        add_dep_helper(dm.ins, after.ins, reason="absorb order")
    return dm


def _alloc_head(tc, pools, h):
    nc = tc.nc
    (trans, vpool, *_rest) = pools
    qTs = trans.tile([128, NSEQ], BF16, tag="qTs")
    kTp = trans.tile([128, NSEQ + 128], BF16, tag="kTp")
    vsh = vpool.tile([128, NT + 1, R, D + 1], BF16, tag="vsh")

    # kTp edge zeros + vsh edges/ones on DVE (idle at startup; gpsimd is
    # busy with mask init and must not serialize any DMA path)
    nc.vector.memset(kTp[:, 0:HALF], 0.0)
    nc.vector.memset(kTp[:, NSEQ + HALF :], 0.0)
    nc.vector.memset(vsh[0:64, 0, :, 0:D], 0.0)
    nc.vector.memset(vsh[64:128, NT, :, 0:D], 0.0)
    nc.vector.memset(vsh[:, :, :, D], 1.0)
    return qTs, kTp, vsh


def _load_head_qk(tc, qTs, kTp, qd, kd, h):
    """Xbar transposes, first-needed first (halves so compute can begin after
    the first pair lands)."""
    nc = tc.nc
    hq = NSEQ // 2
    nc.sync.dma_start_transpose(qTs[:, 0:hq], qd[h, 0:hq, :])
    nc.sync.dma_start_transpose(kTp[:, HALF : HALF + hq], kd[h, 0:hq, :])
    nc.sync.dma_start_transpose(qTs[:, hq:], qd[h, hq:, :])
    nc.sync.dma_start_transpose(kTp[:, HALF + hq : HALF + NSEQ], kd[h, hq:, :])


def _load_head_v(tc, vsh, vd, h):
    """vsh: partitions 64:128 <- tile-aligned rows, 0:64 <- next tile's rows
    (split by dilation offset r to keep each AP 3-dim for the balancer)."""
    nc = tc.nc
    for r in range(R):
        nc.sync.dma_start(
            vsh[64:128, 0:NT, r, 0:D],
            vd[h, :, 0:64, r].rearrange("t pp d -> pp t d"),
        )
        nc.sync.dma_start(
            vsh[0:64, 1 : NT + 1, r, 0:D],
            vd[h, :, 64:128, r].rearrange("t pp d -> pp t d"),
        )


def _qk_phase(tc, pools, masks, qTs, kTp, dmy, h, prev_last_pe):
    """QK scores + exp + band mask for one head. Returns (last_pe, pts)."""
    nc = tc.nc
    (trans, vpool, ppool, opool, rpool, ps_pool, po_pool, dpool) = pools
    m_first_big, m_mid_sm, m_mid_big, m_last_big = masks
    hq = NSEQ // 2
    dc = 8 * (h % 2)
    last_pe = prev_last_pe
    dm_gate = None
    pms = []

    for g in range(NT // 2):
        if g == 0:
            dm_gate = _dummy(
                nc, dmy, dc + 0, qTs[0:1, 0:1], qTs[0:1, HALF : HALF + 1],
                after=last_pe,
            )
            dm_gate = _dummy(
                nc, dmy, dc + 1, kTp[0:1, HALF : HALF + 1],
                kTp[0:1, HALF + 1 : HALF + 2], after=dm_gate,
            )
        elif g == NT // 4:
            dm_gate = _dummy(
                nc, dmy, dc + 2, qTs[0:1, hq : hq + 1],
                kTp[0:1, HALF + hq : HALF + hq + 1], after=last_pe,
            )

        ps = ps_pool.tile([128, 2, 4, 128], F32, tag="ps")
        q0 = 256 * g
        for blk, (p0, p1) in enumerate(((0, 64), (64, 128))):
            mm = nc.tensor.matmul(
                ps[:, blk, 0, :],
                lhsT=kTp[p0:p1, q0 : q0 + 128],
                rhs=qTs[p0:p1, q0 : q0 + 128],
                start=True,
                stop=True,
            )
            if dm_gate is not None:
                add_dep_helper(mm.ins, dm_gate.ins, reason="absorb order")
                dm_gate = None
            nc.tensor.matmul(
                ps[:, blk, 1:3, :],
                lhsT=kTp[p0:p1, q0 + 128 : q0 + 256],
                rhs=qTs[p0:p1, q0 : q0 + 256],
                start=True,
                stop=True,
            )
            last_pe = nc.tensor.matmul(
                ps[:, blk, 3, :],
                lhsT=kTp[p0:p1, q0 + 256 : q0 + 384],
                rhs=qTs[p0:p1, q0 + 128 : q0 + 256],
                start=True,
                stop=True,
            )

        # exp((q.k)/8) for both tiles in one ScalarE pass; bf16 out.
        pt = ppool.tile([128, 2, 4, 128], BF16, tag="pt")
        nc.scalar.activation(
            pt[:], ps[:], mybir.ActivationFunctionType.Exp, scale=1.0 / float(D) ** 0.5
        )

        # band mask (0/1 multiply): lo segs keep i<=jj, hi segs keep i>=jj;
        # first/last tiles additionally kill out-of-range keys. Runs on
        # gpsimd (all-SBUF op; Pool is otherwise idle) so DVE is free to
        # chase the PV chain with reciprocal+normalize — otherwise the
        # stores queue behind the other head's mask chain on DVE.
        pm = ppool.tile([128, 2, 4, 128], BF16, tag="pm")
        if 0 < g < NT // 2 - 1:
            nc.gpsimd.tensor_tensor(
                pm[:].rearrange("p b (j c) i -> p (b j) c i", c=2),
                pt[:].rearrange("p b (j c) i -> p (b j) c i", c=2),
                m_mid_big[:],
                mybir.AluOpType.mult,
            )
        else:
            for j, t in enumerate((2 * g, 2 * g + 1)):
                m = (
                    m_first_big
                    if t == 0
                    else (m_last_big if t == NT - 1 else m_mid_sm)
                )
                nc.gpsimd.tensor_tensor(
                    pm[:, :, 2 * j : 2 * j + 2, :],
                    pt[:, :, 2 * j : 2 * j + 2, :],
                    m[:],
                    mybir.AluOpType.mult,
                )
        pms.append(pm)
    return last_pe, pms


def _pv_phase(tc, pools, vsh, pms, out, dmy, h, prev_last_pe):
    """PV + normalize + store for one head. Pinned after the V-load gates."""
    nc = tc.nc
    (trans, vpool, ppool, opool, rpool, ps_pool, po_pool, dpool) = pools
    dc = 8 * (h % 2)
    out_sb = opool.tile([128, NT, 128], F32, tag="out_sb")
    last_pe = prev_last_pe

    dm_gate = _dummy(
        nc, dmy, dc + 4, vsh[64:65, 0, 0, 0:1], vsh[64:65, 0, 1, 0:1],
        after=last_pe,
    )
    dm_gate = _dummy(
        nc, dmy, dc + 5, vsh[0:1, 1, 0, 0:1], vsh[0:1, 1, 1, 0:1],
        after=dm_gate,
    )

    for g in range(NT // 2):
        pm = pms[g]
        po = po_pool.tile([128, 2, 2, D + 1], F32, tag="po")
        for j, t in enumerate((2 * g, 2 * g + 1)):
            for blk in range(R):
                mm = nc.tensor.matmul(
                    po[:, j, blk, :],
                    lhsT=pm[:, blk, 2 * j + 0, :],
                    rhs=vsh[:, t, blk, :],
                    start=True,
                    stop=False,
                )
                if dm_gate is not None:
                    add_dep_helper(mm.ins, dm_gate.ins, reason="absorb order")
                    dm_gate = None
                last_pe = nc.tensor.matmul(
                    po[:, j, blk, :],
                    lhsT=pm[:, blk, 2 * j + 1, :],
                    rhs=vsh[:, t + 1, blk, :],
                    start=False,
                    stop=True,
                )
        # normalize both tiles at once: out = po[..., 0:64] / po[..., 64]
        rc = rpool.tile([128, 2, 2], F32, tag="rc")
        nc.vector.reciprocal(rc[:], po[:, :, :, D])
        nc.vector.tensor_tensor(
            out_sb[:, 2 * g : 2 * g + 2, :].rearrange("p t (r d) -> p t r d", r=R),
            po[:, :, :, 0:D],
            rc[:, :, :, None].to_broadcast((128, 2, R, D)),
            mybir.AluOpType.mult,
        )

    # single store per head (a store interleaved with the transposes would
    # serialize against them; by PV time the xbar chain has drained)
    nc.sync.dma_start(
        out[h].rearrange("(t p r) d -> p t (r d)", p=128, r=R),
        out_sb[:],
    )
    return last_pe


def _build_masks(tc, mpool):
    """Band masks [128, 2(lo|hi), 128].

    Element [jj, c, i]: lo (c=0) keeps i <= jj, hi (c=1) keeps i >= jj.
    t=0 variant also kills jj < 64 in lo (keys < 0); t=NT-1 variant kills
    jj >= 64 in hi (keys >= NSEQ).
    """
    nc = tc.nc
    ge = mybir.AluOpType.is_ge
    tiles = []
    for name in ("m_first", "m_mid", "m_last"):
        m = mpool.tile([128, 2, 128], BF16, tag=name)
        nc.gpsimd.memset(m[:], 1.0)
        # lo: keep jj - i >= 0
        nc.gpsimd.affine_select(
            m[:, 0, :], m[:, 0, :], [[-1, 128]], ge, 0.0,
            base=0, channel_multiplier=1,
        )
        # hi: keep i - jj >= 0
        nc.gpsimd.affine_select(
            m[:, 1, :], m[:, 1, :], [[1, 128]], ge, 0.0,
            base=0, channel_multiplier=-1,
        )
        tiles.append(m)
    m_first, m_mid, m_last = tiles
    # first tile: lo also needs jj >= 64
    nc.gpsimd.affine_select(
        m_first[:, 0, :], m_first[:, 0, :], [[0, 128]], ge, 0.0,
        base=-HALF, channel_multiplier=1,
    )
    # last tile: hi also needs jj <= 63
    nc.gpsimd.affine_select(
        m_last[:, 1, :], m_last[:, 1, :], [[0, 128]], ge, 0.0,
        base=HALF - 1, channel_multiplier=-1,
    )

    # Materialize broadcast copies (flat operands keep the Pool TT at full
    # rate; the stride-0 broadcast halves it). Mask multiplies run on
    # gpsimd too, so same-engine program order covers this init and each
    # mask op carries only its exp wait.
    m_first_big = mpool.tile([128, 2, 2, 128], BF16, tag="m_first_big")
    m_mid_sm = mpool.tile([128, 2, 2, 128], BF16, tag="m_mid_sm")
    m_mid_big = mpool.tile([128, 4, 2, 128], BF16, tag="m_mid_big")
    m_last_big = mpool.tile([128, 2, 2, 128], BF16, tag="m_last_big")
    for big, base, nb in (
        (m_mid_big, m_mid, 4),
        (m_first_big, m_first, 2),
        (m_mid_sm, m_mid, 2),
        (m_last_big, m_last, 2),
    ):
        nc.gpsimd.tensor_copy(
            big[:], base[:, None, :, :].to_broadcast((128, nb, 2, 128))
        )
    return m_first_big, m_mid_sm, m_mid_big, m_last_big


def build_bass():
    nc = bacc.Bacc("TRN2", target_bir_lowering=False, debug=False)
    qd = nc.dram_tensor("q", [HPC, NSEQ, R * D], BF16, kind="ExternalInput")
    kd = nc.dram_tensor("k", [HPC, NSEQ, R * D], BF16, kind="ExternalInput")
    vd = nc.dram_tensor("v", [HPC, NT, 128, R, D], BF16, kind="ExternalInput")
    out = nc.dram_tensor("out", [HPC, S, D], F32, kind="ExternalOutput")

    with tile.TileContext(nc) as tc:
        with (
            tc.tile_pool(name="mpool", bufs=1) as mpool,
            tc.tile_pool(name="trans", bufs=2) as trans,
            tc.tile_pool(name="vpool", bufs=2) as vpool,
            tc.tile_pool(name="ppool", bufs=4 * (NT // 2)) as ppool,
            tc.tile_pool(name="opool", bufs=2) as opool,
            tc.tile_pool(name="rpool", bufs=8) as rpool,
            tc.tile_pool(name="ps_pool", bufs=2, space="PSUM") as ps_pool,
            tc.tile_pool(name="po_pool", bufs=3, space="PSUM") as po_pool,
            tc.tile_pool(name="dmy_pool", bufs=1, space="PSUM") as dpool,
        ):
            masks = _build_masks(tc, mpool)
            pools = (trans, vpool, ppool, opool, rpool, ps_pool, po_pool, dpool)
            tiles = [_alloc_head(tc, pools, h) for h in range(HPC)]
            # xbar chain first (it serializes against all other DMA), then
            # the V loads ride behind it on the same queue
            for h in range(HPC):
                _load_head_qk(tc, tiles[h][0], tiles[h][1], qd, kd, h)
            for h in range(HPC):
                _load_head_v(tc, tiles[h][2], vd, h)
            dmy = dpool.tile([1, 16], F32, tag="dmy")
            last_pe = None
            all_pms = []
            for h in range(HPC):
                qTs, kTp, vsh = tiles[h]
                last_pe, pms = _qk_phase(
                    tc, pools, masks, qTs, kTp, dmy, h, last_pe
                )
                all_pms.append(pms)
            for h in range(HPC):
                qTs, kTp, vsh = tiles[h]
                last_pe = _pv_phase(
                    tc, pools, vsh, all_pms[h], out[:], dmy, h, last_pe
                )
    nc.compile()
    return nc


_NC_CACHE = None


def kernel(q: np.ndarray, k: np.ndarray, v: np.ndarray) -> np.ndarray:
    global _NC_CACHE
    if _NC_CACHE is None:
        _NC_CACHE = build_bass()
    nc = _NC_CACHE

    # host side: one bf16 cast per tensor; all relayout is on-device
    qb = np.asarray(q, dtype=np.float32).reshape(H, NSEQ, R * D).astype(
        ml_dtypes.bfloat16
    )
    kb = np.asarray(k, dtype=np.float32).reshape(H, NSEQ, R * D).astype(
        ml_dtypes.bfloat16
    )
    vb = np.asarray(v, dtype=np.float32).reshape(H, NT, 128, R, D).astype(
        ml_dtypes.bfloat16
    )

    in_maps = []
    for c in range(N_CORES):
        hs = slice(c * HPC, (c + 1) * HPC)
        in_maps.append({"q": qb[hs], "k": kb[hs], "v": vb[hs]})

    res = run_bass_kernel_spmd(nc, in_maps, core_ids=list(range(N_CORES)))
    out = np.empty((B, H, S, D), dtype=np.float32)
    for c in range(N_CORES):
        out[0, c * HPC : (c + 1) * HPC] = res.results[c]["out"]
    return out


# revision 37
# speedup vs baseline: 1.2871x; 1.2871x over previous
"""Dilated attention (banded local-window attention) for Trainium2.

Problem: q,k,v [1, 16, 4096, 64] fp32; dilation r=2, window 128 (band |i-j|<=64
within each of the 2 strided subsequences of length 2048 per head).

Sharding: 16 heads -> 8 cores x 2 heads. Host work is a single fp32->bf16 cast
per tensor (ml_dtypes, ~10ms each); ALL relayout happens on device:

- Q^T/K^T: xbar DMA-transposes turn DRAM-natural [2048, (r d)=128] bf16 into
  SBUF [(r d), 2048] d-major layout; offset r=0 lands on partitions 0:64 and
  r=1 on 64:128 (the dilation de-interleave falls out of the transpose since
  (r d) becomes the partition dim). K^T lands at column offset 64 inside a
  [128, 2176] tile whose edge columns are zeroed. The transposes are SP-issued
  (ACT-issued DmaTranspose corrupts on this runtime) and serialize against
  all other DMA traffic, so they run as one tight chain before the V loads.
- V: the shifted window layout vsh[p, t, r, :64] = v[2*(128t - 64 + p) + r]
  (+ ones column for the row-sum trick) is built with strided DMAs plus edge
  memsets.

Per block, queries are tiled in 16 tiles of 128; each tile attends to a
256-key window. Scores are computed transposed (S^T[jj, i]) so probabilities
come out pre-transposed for the PV matmul. Softmax skips the max-subtraction
(scores ~ N(0,1) after the 1/8 scale) and folds the scale into the ScalarE
exp. The band mask is a 0/1 bf16 multiply after exp. Row sums come from the
ones-column: out = (P@[V|1])[:,:64] / (P@[V|1])[:,64].

Emission is phase-split so the ScalarE exp chain (the serial bottleneck)
never stalls: QK+exp+mask for both heads runs while V still loads; all PV
work is pinned after the V-load gates.
"""

import sys

for _p in ("/opt/trn_rl_repo", "/opt/trn_rl_repo/concourse"):
    if _p not in sys.path:
        sys.path.insert(0, _p)

import numpy as np
import ml_dtypes

import concourse.bass as bass
import concourse.mybir as mybir
import concourse.tile as tile
from concourse import bacc
from concourse.bass_utils import run_bass_kernel_spmd
from concourse.tile_rust import add_dep_helper

N_CORES = 8
B, H, S, D = 1, 16, 4096, 64
R = 2                      # dilation rate
NSEQ = S // R              # 2048 per-offset sequence length
HALF = 64                  # window//2
NT = NSEQ // 128           # 16 query tiles per block
HPC = H // N_CORES         # heads per core = 2

F32 = mybir.dt.float32
BF16 = mybir.dt.bfloat16


def _dummy(nc, dmy, col, a, b, after=None):
    """One PE-proc absorber: a tiny dummy matmul reading cells a/b so the PE
    sequencer observes their DMA-queue semaphores here (<=2 waits). The
    scheduler floats unpinned dummies past the real matmuls (leaving the DMA
    waits guarding nothing), so callers must pin consumers after these via
    add_dep_helper, and `after` keeps the dummy from stalling the PE early."""
    dm = nc.tensor.matmul(dmy[0:1, col : col + 1], lhsT=a, rhs=b,
                          start=True, stop=True)
    if after is not None:
        add_dep_helper(dm.ins, after.ins, reason="absorb order")
    return dm


def _alloc_head(tc, pools, h):
    nc = tc.nc
    (trans, vpool, *_rest) = pools
    qTs = trans.tile([128, NSEQ], BF16, tag="qTs")
    kTp = trans.tile([128, NSEQ + 128], BF16, tag="kTp")
    vsh = vpool.tile([128, NT + 1, R, D + 1], BF16, tag="vsh")

    # kTp edge zeros + vsh edges/ones on DVE (idle at startup; gpsimd is
    # busy with mask init and must not serialize any DMA path)
    nc.vector.memset(kTp[:, 0:HALF], 0.0)
    nc.vector.memset(kTp[:, NSEQ + HALF :], 0.0)
    nc.vector.memset(vsh[0:64, 0, :, 0:D], 0.0)
    nc.vector.memset(vsh[64:128, NT, :, 0:D], 0.0)
    nc.vector.memset(vsh[:, :, :, D], 1.0)
    return qTs, kTp, vsh


def _load_head_qk(tc, qTs, kTp, qd, kd, h):
    """Xbar transposes, first-needed first (split so compute can begin after
    the first pair lands; h0's first piece is small to start the pipeline
    as early as possible)."""
    nc = tc.nc
    sp = 512 if h == 0 else NSEQ // 2
    nc.sync.dma_start_transpose(qTs[:, 0:sp], qd[h, 0:sp, :])
    nc.sync.dma_start_transpose(kTp[:, HALF : HALF + sp], kd[h, 0:sp, :])
    nc.sync.dma_start_transpose(qTs[:, sp:], qd[h, sp:, :])
    nc.sync.dma_start_transpose(kTp[:, HALF + sp : HALF + NSEQ], kd[h, sp:, :])


def _load_head_v(tc, vsh, vd, h):
    """vsh: partitions 64:128 <- tile-aligned rows, 0:64 <- next tile's rows
    (split by dilation offset r to keep each AP 3-dim for the balancer)."""
    nc = tc.nc
    for r in range(R):
        nc.sync.dma_start(
            vsh[64:128, 0:NT, r, 0:D],
            vd[h, :, 0:64, r].rearrange("t pp d -> pp t d"),
        )
        nc.sync.dma_start(
            vsh[0:64, 1 : NT + 1, r, 0:D],
            vd[h, :, 64:128, r].rearrange("t pp d -> pp t d"),
        )


def _qk_phase(tc, pools, masks, qTs, kTp, dmy, h, prev_last_pe):
    """QK scores + exp + band mask for one head. Returns (last_pe, pts)."""
    nc = tc.nc
    (trans, vpool, ppool, opool, rpool, ps_pool, po_pool, dpool) = pools
    m_first, m_mid, m_last = masks
    # xbar split point for this head: h0 gets a small first piece so its
    # first QK group starts as early as possible; h1's halves land well
    # before its compute begins anyway
    split = 512 if h == 0 else NSEQ // 2
    dc = 8 * (h % 2)
    last_pe = prev_last_pe
    dm_gate = None
    pms = []

    for g in range(NT // 2):
        if g == 0:
            dm_gate = _dummy(
                nc, dmy, dc + 0, qTs[0:1, 0:1], qTs[0:1, HALF : HALF + 1],
                after=last_pe,
            )
            dm_gate = _dummy(
                nc, dmy, dc + 1, kTp[0:1, HALF : HALF + 1],
                kTp[0:1, HALF + 1 : HALF + 2], after=dm_gate,
            )
        elif g == split // 256:
            dm_gate = _dummy(
                nc, dmy, dc + 2, qTs[0:1, split : split + 1],
                kTp[0:1, HALF + split : HALF + split + 1], after=last_pe,
            )

        ps = ps_pool.tile([128, 2, 4, 128], F32, tag="ps")
        q0 = 256 * g
        for blk, (p0, p1) in enumerate(((0, 64), (64, 128))):
            mm = nc.tensor.matmul(
                ps[:, blk, 0, :],
                lhsT=kTp[p0:p1, q0 : q0 + 128],
                rhs=qTs[p0:p1, q0 : q0 + 128],
                start=True,
                stop=True,
            )
            if dm_gate is not None:
                add_dep_helper(mm.ins, dm_gate.ins, reason="absorb order")
                dm_gate = None
            nc.tensor.matmul(
                ps[:, blk, 1:3, :],
                lhsT=kTp[p0:p1, q0 + 128 : q0 + 256],
                rhs=qTs[p0:p1, q0 : q0 + 256],
                start=True,
                stop=True,
            )
            last_pe = nc.tensor.matmul(
                ps[:, blk, 3, :],
                lhsT=kTp[p0:p1, q0 + 256 : q0 + 384],
                rhs=qTs[p0:p1, q0 + 128 : q0 + 256],
                start=True,
                stop=True,
            )

        # exp((q.k)/8) for both tiles in one ScalarE pass; bf16 out.
        pt = ppool.tile([128, 2, 4, 128], BF16, tag="pt")
        nc.scalar.activation(
            pt[:], ps[:], mybir.ActivationFunctionType.Exp, scale=1.0 / float(D) ** 0.5
        )

        # band mask (0/1 multiply): lo segs keep i<=jj, hi segs keep i>=jj;
        # first/last tiles additionally kill out-of-range keys. Runs on
        # gpsimd (all-SBUF op; Pool is otherwise idle) so DVE is free to
        # chase the PV chain with reciprocal+normalize — otherwise the
        # stores queue behind the other head's mask chain on DVE.
        pm = ppool.tile([128, 2, 4, 128], BF16, tag="pm")
        if 0 < g < NT // 2 - 1:
            nc.vector.tensor_tensor(
                pm[:].rearrange("p b (j c) i -> p (b j) c i", c=2),
                pt[:].rearrange("p b (j c) i -> p (b j) c i", c=2),
                m_mid[:, None, :, :].to_broadcast((128, 4, 2, 128)),
                mybir.AluOpType.mult,
            )
        else:
            for j, t in enumerate((2 * g, 2 * g + 1)):
                m = m_first if t == 0 else (m_last if t == NT - 1 else m_mid)
                nc.vector.tensor_tensor(
                    pm[:, :, 2 * j : 2 * j + 2, :],
                    pt[:, :, 2 * j : 2 * j + 2, :],
                    m[:, None, :, :].to_broadcast((128, 2, 2, 128)),
                    mybir.AluOpType.mult,
                )
        pms.append(pm)
    return last_pe, pms


def _pv_phase(tc, pools, vsh, pms, out, dmy, h, prev_last_pe):
    """PV + normalize + store for one head. Pinned after the V-load gates."""
    nc = tc.nc
    (trans, vpool, ppool, opool, rpool, ps_pool, po_pool, dpool) = pools
    dc = 8 * (h % 2)
    out_sb = opool.tile([128, NT, 128], F32, tag="out_sb")
    last_pe = prev_last_pe

    dm_gate = _dummy(
        nc, dmy, dc + 4, vsh[64:65, 0, 0, 0:1], vsh[64:65, 0, 1, 0:1],
        after=last_pe,
    )
    dm_gate = _dummy(
        nc, dmy, dc + 5, vsh[0:1, 1, 0, 0:1], vsh[0:1, 1, 1, 0:1],
        after=dm_gate,
    )

    for g in range(NT // 2):
        pm = pms[g]
        po = po_pool.tile([128, 2, 2, D + 1], F32, tag="po")
        for j, t in enumerate((2 * g, 2 * g + 1)):
            for blk in range(R):
                mm = nc.tensor.matmul(
                    po[:, j, blk, :],
                    lhsT=pm[:, blk, 2 * j + 0, :],
                    rhs=vsh[:, t, blk, :],
                    start=True,
                    stop=False,
                )
                if dm_gate is not None:
                    add_dep_helper(mm.ins, dm_gate.ins, reason="absorb order")
                    dm_gate = None
                last_pe = nc.tensor.matmul(
                    po[:, j, blk, :],
                    lhsT=pm[:, blk, 2 * j + 1, :],
                    rhs=vsh[:, t + 1, blk, :],
                    start=False,
                    stop=True,
                )
        # normalize both tiles at once: out = po[..., 0:64] / po[..., 64]
        rc = rpool.tile([128, 2, 2], F32, tag="rc")
        nc.vector.reciprocal(rc[:], po[:, :, :, D])
        nc.vector.tensor_tensor(
            out_sb[:, 2 * g : 2 * g + 2, :].rearrange("p t (r d) -> p t r d", r=R),
            po[:, :, :, 0:D],
            rc[:, :, :, None].to_broadcast((128, 2, R, D)),
            mybir.AluOpType.mult,
        )

    # single store per head (a store interleaved with the transposes would
    # serialize against them; by PV time the xbar chain has drained)
    nc.sync.dma_start(
        out[h].rearrange("(t p r) d -> p t (r d)", p=128, r=R),
        out_sb[:],
    )
    return last_pe


def _build_masks(tc, mpool):
    """Band masks [128, 2(lo|hi), 128].

    Element [jj, c, i]: lo (c=0) keeps i <= jj, hi (c=1) keeps i >= jj.
    t=0 variant also kills jj < 64 in lo (keys < 0); t=NT-1 variant kills
    jj >= 64 in hi (keys >= NSEQ).
    """
    nc = tc.nc
    ge = mybir.AluOpType.is_ge
    tiles = []
    for name in ("m_first", "m_mid", "m_last"):
        m = mpool.tile([128, 2, 128], BF16, tag=name)
        nc.gpsimd.memset(m[:], 1.0)
        # lo: keep jj - i >= 0
        nc.gpsimd.affine_select(
            m[:, 0, :], m[:, 0, :], [[-1, 128]], ge, 0.0,
            base=0, channel_multiplier=1,
        )
        # hi: keep i - jj >= 0
        nc.gpsimd.affine_select(
            m[:, 1, :], m[:, 1, :], [[1, 128]], ge, 0.0,
            base=0, channel_multiplier=-1,
        )
        tiles.append(m)
    m_first, m_mid, m_last = tiles
    # first tile: lo also needs jj >= 64
    nc.gpsimd.affine_select(
        m_first[:, 0, :], m_first[:, 0, :], [[0, 128]], ge, 0.0,
        base=-HALF, channel_multiplier=1,
    )
    # last tile: hi also needs jj <= 63
    nc.gpsimd.affine_select(
        m_last[:, 1, :], m_last[:, 1, :], [[0, 128]], ge, 0.0,
        base=HALF - 1, channel_multiplier=-1,
    )

    # DVE-proc absorber: make the DVE clock observe the final Pool init op
    # here so the first real mask multiply carries only its exp wait (the
    # TensorTensor ISA struct has a single sync-wait slot).
    mdmy = mpool.tile([1, 2], BF16, tag="mdmy")
    nc.vector.tensor_tensor(
        mdmy[0:1, 0:1], m_last[0:1, 0, 0:1], m_last[0:1, 1, 0:1],
        mybir.AluOpType.mult,
    )
    return m_first, m_mid, m_last


def build_bass():
    nc = bacc.Bacc("TRN2", target_bir_lowering=False, debug=False)
    qd = nc.dram_tensor("q", [HPC, NSEQ, R * D], BF16, kind="ExternalInput")
    kd = nc.dram_tensor("k", [HPC, NSEQ, R * D], BF16, kind="ExternalInput")
    vd = nc.dram_tensor("v", [HPC, NT, 128, R, D], BF16, kind="ExternalInput")
    out = nc.dram_tensor("out", [HPC, S, D], F32, kind="ExternalOutput")

    with tile.TileContext(nc) as tc:
        with (
            tc.tile_pool(name="mpool", bufs=1) as mpool,
            tc.tile_pool(name="trans", bufs=2) as trans,
            tc.tile_pool(name="vpool", bufs=2) as vpool,
            tc.tile_pool(name="ppool", bufs=4 * (NT // 2)) as ppool,
            tc.tile_pool(name="opool", bufs=2) as opool,
            tc.tile_pool(name="rpool", bufs=8) as rpool,
            tc.tile_pool(name="ps_pool", bufs=2, space="PSUM") as ps_pool,
            tc.tile_pool(name="po_pool", bufs=3, space="PSUM") as po_pool,
            tc.tile_pool(name="dmy_pool", bufs=1, space="PSUM") as dpool,
        ):
            masks = _build_masks(tc, mpool)
            pools = (trans, vpool, ppool, opool, rpool, ps_pool, po_pool, dpool)
            tiles = [_alloc_head(tc, pools, h) for h in range(HPC)]
            # xbar chain first (it serializes against all other DMA), then
            # the V loads ride behind it on the same queue
            for h in range(HPC):
                _load_head_qk(tc, tiles[h][0], tiles[h][1], qd, kd, h)
            for h in range(HPC):
                _load_head_v(tc, tiles[h][2], vd, h)
            dmy = dpool.tile([1, 16], F32, tag="dmy")
            last_pe = None
            all_pms = []
            for h in range(HPC):
                qTs, kTp, vsh = tiles[h]
                last_pe, pms = _qk_phase(
                    tc, pools, masks, qTs, kTp, dmy, h, last_pe
                )
                all_pms.append(pms)
            for h in range(HPC):
                qTs, kTp, vsh = tiles[h]
                last_pe = _pv_phase(
                    tc, pools, vsh, all_pms[h], out[:], dmy, h, last_pe
                )
    nc.compile()
    return nc


_NC_CACHE = None


def kernel(q: np.ndarray, k: np.ndarray, v: np.ndarray) -> np.ndarray:
    global _NC_CACHE
    if _NC_CACHE is None:
        _NC_CACHE = build_bass()
    nc = _NC_CACHE

    # host side: one bf16 cast per tensor; all relayout is on-device
    qb = np.asarray(q, dtype=np.float32).reshape(H, NSEQ, R * D).astype(
        ml_dtypes.bfloat16
    )
    kb = np.asarray(k, dtype=np.float32).reshape(H, NSEQ, R * D).astype(
        ml_dtypes.bfloat16
    )
    vb = np.asarray(v, dtype=np.float32).reshape(H, NT, 128, R, D).astype(
        ml_dtypes.bfloat16
    )

    in_maps = []
    for c in range(N_CORES):
        hs = slice(c * HPC, (c + 1) * HPC)
        in_maps.append({"q": qb[hs], "k": kb[hs], "v": vb[hs]})

    res = run_bass_kernel_spmd(nc, in_maps, core_ids=list(range(N_CORES)))
    out = np.empty((B, H, S, D), dtype=np.float32)
    for c in range(N_CORES):
        out[0, c * HPC : (c + 1) * HPC] = res.results[c]["out"]
    return out
